# revision 8
# baseline (speedup 1.0000x reference)
"""Trainium2 Bass kernel for a causal multi-head attention block.

Problem: y = MHA(x), B=256, N=256 (seq), C=512, H=8 heads, d=64
  Q = x@Wq + bq ; K = x@Wk + bk ; V = x@Wv + bv   (per-head split)
  S = Q K^T ; scaled = (S + causal_mask*-1e5)/sqrt(d) ; P = softmax(scaled)
  y = (P V merged) @ Wo + bo

Sharding: pure data-parallel over batch B across 8 NeuronCores (32 batch
elements per core); weights replicated; no collectives.

Host-side marshalling inside kernel():
 - x is pre-transposed to xT [b, C, N] and cast to bf16.
 - weights pre-cast to bf16.
 - bv/bo folded on host: softmax rows sum to 1, so V's bias contributes
   attn@(1 bv^T) = bv exactly, hence y += bv@Wo + bo after gathering.

Device pipeline (transpose-free attention):
 - QT/KT [C, 2N] = W^T @ xT computed per batch-element pair; bq/bk fused
   into the PSUM->SBUF evacuation (ACT Identity + per-partition bias).
 - V [N, C] = x @ Wv per element (DVE evac).
 - Scores are computed TRANSPOSED: ST[k, q] = K_h^T q stored with keys on
   partitions, by swapping the matmul operands (lhsT=K_h, rhs=Q_h). Only
   the 3 causally-live 128x128 blocks are computed:
     blk0 = (k0, q0), blk1 = (k0, q1), blk2 = (k1, q1).
   Two heads run concurrently in disjoint 64-row PE groups.
 - exp via one ACT op per head (scale=1/8, PSUM->SBUF bf16, no max-sub:
   scores are O(1)). Causal masking = multiply the two diagonal blocks by
   a 0/1 lower-triangular bf16 constant on DVE (masked reference lanes
   underflow exp->0 exactly, so 0 matches bit-for-bit).
 - Z[q] (softmax denominators) via ones-matmul column sums on the PE:
   zpair[64h..64h+64, q] = ones64^T @ expST_h, head pairs col-packed.
 - attnT_raw [d, q] = V_h^T @ expST_h straight into the [C, N] layout
   (col-packed pairs). Normalization happens on the small attnT tile:
   at_sb = at_ps * reciprocal(zpair) on DVE (1/Z broadcast over d rows
   comes for free since zpair rows are all identical per head).
 - y [N, C] = attnT^T @ Wo, DVE evac, DMA out.

No PE transposes anywhere -> tensor engine stays HAM-warm and does only
real matmul work.
"""

import os
import sys

sys.path.insert(0, "/opt/trn_rl_repo")

import numpy as np

import concourse.bass as bass
import concourse.mybir as mybir
import concourse.tile as tile
from concourse import bacc

B, N, C, H, D = 256, 256, 512, 8, 64
NCORES = 8
NB = B // NCORES  # batch elements per core
P = 128
F32 = mybir.dt.float32

AF = mybir.ActivationFunctionType

MM_DT = mybir.dt.bfloat16
PV_DT = mybir.dt.bfloat16


def _emit(nc: bass.Bass, nb: int):
    xt_in = nc.dram_tensor("xt", [nb, C, N], MM_DT, kind="ExternalInput")
    Wq = nc.dram_tensor("Wq", [C, C], MM_DT, kind="ExternalInput")
    Wk = nc.dram_tensor("Wk", [C, C], MM_DT, kind="ExternalInput")
    Wv = nc.dram_tensor("Wv", [C, C], MM_DT, kind="ExternalInput")
    Wo = nc.dram_tensor("Wo", [C, C], MM_DT, kind="ExternalInput")
    bq = nc.dram_tensor("bq", [C], F32, kind="ExternalInput")
    bk = nc.dram_tensor("bk", [C], F32, kind="ExternalInput")
    y = nc.dram_tensor("y", [nb, N, C], F32, kind="ExternalOutput")

    CB = C // P  # 4 column blocks of 128
    TB = N // P  # 2 row blocks of 128

    with (
        tile.TileContext(nc) as tc,
        tc.tile_pool(name="consts", bufs=1) as consts,
        tc.tile_pool(name="io", bufs=3) as io,
        tc.tile_pool(name="work", bufs=3) as work,
        tc.tile_pool(name="heads", bufs=int(os.environ.get("HB", "4"))) as heads,
        tc.tile_pool(name="ps_mm", bufs=int(os.environ.get("PS_MM", "2")), space="PSUM") as ps_mm,
        tc.tile_pool(name="ps_sc", bufs=int(os.environ.get("PS_SC", "2")), space="PSUM") as ps_sc,
        tc.tile_pool(name="ps_zat", bufs=int(os.environ.get("PS_ZAT", "2")), space="PSUM") as ps_zat,
    ):
        # ---- constants ----
        # keep-mask for the two causal diagonal blocks of both heads of a
        # pair: 1 where k <= q (p >= f), else 0.  Layout [P, sub, blk, P].
        mk4 = consts.tile([P, 2, 2, P], PV_DT)
        nc.gpsimd.memset(mk4, 1.0)
        for i in range(2):
            for j in range(2):
                nc.gpsimd.affine_select(
                    out=mk4[:, i, j, :],
                    in_=mk4[:, i, j, :],
                    compare_op=mybir.AluOpType.is_ge,
                    fill=0.0,
                    base=0,
                    pattern=[[1, P]],
                    channel_multiplier=-1,
                )
        ones64 = consts.tile([P, D], PV_DT)
        nc.gpsimd.memset(ones64, 1.0)

        wq_sb = consts.tile([P, CB, C], MM_DT)
        nc.sync.dma_start(wq_sb, Wq.rearrange("(k p) m -> p k m", p=P))
        wk_sb = consts.tile([P, CB, C], MM_DT)
        nc.sync.dma_start(wk_sb, Wk.rearrange("(k p) m -> p k m", p=P))
        wv_sb = consts.tile([P, CB, C], MM_DT)
        nc.sync.dma_start(wv_sb, Wv.rearrange("(k p) m -> p k m", p=P))
        wo_sb = consts.tile([P, CB, C], MM_DT)
        nc.sync.dma_start(wo_sb, Wo.rearrange("(k p) m -> p k m", p=P))
        bq_sb = consts.tile([P, CB], F32)
        nc.sync.dma_start(bq_sb, bq.rearrange("(m p) -> p m", p=P))
        bk_sb = consts.tile([P, CB], F32)
        nc.sync.dma_start(bk_sb, bk.rearrange("(m p) -> p m", p=P))

        for pi in range(nb // 2):
            # ---- load pair of batch elems (already transposed on host) ----
            xT = work.tile([P, CB, 2 * N], MM_DT, tag="xT")
            for e in range(2):
                i = pi * 2 + e
                nc.sync.dma_start(
                    xT[:, :, e * N : (e + 1) * N],
                    xt_in[i].rearrange("(cb p) n -> p cb n", p=P),
                )

            # ---- paired projections: QT/KT [C, 2N] = W^T @ xT ----
            qt = work.tile([P, CB, 2 * N], MM_DT, tag="qt")
            kt = work.tile([P, CB, 2 * N], MM_DT, tag="kt")
            for mb in range(CB):
                pq = ps_mm.tile([P, 2 * N], F32, tag="mm")
                for k in range(CB):
                    nc.tensor.matmul(
                        pq,
                        wq_sb[:, k, mb * P : (mb + 1) * P],
                        xT[:, k, :],
                        start=(k == 0),
                        stop=(k == CB - 1),
                    )
                nc.vector.tensor_scalar_add(
                    qt[:, mb, :], pq, bq_sb[:, mb : mb + 1]
                )
                pk = ps_mm.tile([P, 2 * N], F32, tag="mm")
                for k in range(CB):
                    nc.tensor.matmul(
                        pk,
                        wk_sb[:, k, mb * P : (mb + 1) * P],
                        xT[:, k, :],
                        start=(k == 0),
                        stop=(k == CB - 1),
                    )
                nc.vector.tensor_scalar_add(
                    kt[:, mb, :], pk, bk_sb[:, mb : mb + 1]
                )

            # ---- V [N, C] = x @ Wv for both elements of the pair ----
            v_sbs = []
            for e in range(2):
                eo = e * N
                v_sb = work.tile([P, TB, C], PV_DT, tag=f"v{e}")
                for t in range(TB):
                    pv = ps_mm.tile([P, C], F32, tag="mm")
                    for k in range(CB):
                        nc.tensor.matmul(
                            pv,
                            xT[:, k, eo + t * P : eo + (t + 1) * P],
                            wv_sb[:, k, :],
                            start=(k == 0),
                            stop=(k == CB - 1),
                        )
                    nc.vector.tensor_copy(v_sb[:, t, :], pv)
                v_sbs.append(v_sb)

            # ---- attention, transposed scores; elements interleaved so
            # the PE never waits on a single element's softmax tail ----
            at_sbs = [
                work.tile([P, CB, N], MM_DT, tag=f"at{e}", name=f"at_sb{e}")
                for e in range(2)
            ]
            for hp_i in range(H // 2):
                for e in range(2):
                    eo = e * N
                    v_sb = v_sbs[e]
                    # ST blocks for both heads of the pair.
                    # s_pair[:, sub, 0:2, :] = K_h(k0)^T Q_h  (q0 | q1)
                    # s_pair[:, sub, 2, :]   = K_h(k1)^T Q_h  (q1)
                    # sub-tiles sit in separate PSUM banks (4 blks * 512B).
                    s_pair = ps_sc.tile([P, 2, 4, P], F32, tag="sc")
                    for sub in range(2):
                        hp = D * sub
                        qh = qt[hp : hp + D, hp_i, eo : eo + N]
                        kh = kt[hp : hp + D, hp_i, eo : eo + N]
                        nc.tensor.matmul(
                            s_pair[:, sub, 0:2, :], kh[:, 0:P], qh,
                            start=True, stop=True, skip_group_check=True,
                        )
                        nc.tensor.matmul(
                            s_pair[:, sub, 2, :], kh[:, P:N], qh[:, P:N],
                            start=True, stop=True, skip_group_check=True,
                        )
                    # exp((S+mask)/sqrt(d)) without mask: masked lanes are
                    # zeroed right after (reference underflows to 0 too).
                    est = heads.tile([P, 2, 3, P], PV_DT, tag="est")
                    nc.scalar.activation(
                        est, s_pair[:, 0:2, 0:3, :], AF.Exp, scale=0.125
                    )
                    nc.gpsimd.tensor_mul(
                        est[:, 0:2, 0:3:2, :], est[:, 0:2, 0:3:2, :], mk4
                    )
                    # Z column sums (ones-matmul) + attnT_raw, col-packed,
                    # sharing one PSUM bank ([:, 0, :] = attnT, [:, 1, :] = Z).
                    zat = ps_zat.tile([P, 2, N], F32, tag="zat")
                    for sub in range(2):
                        hp = D * sub
                        nc.tensor.matmul(
                            zat[hp : hp + D, 1, :],
                            ones64,
                            est[:, sub, 0:2, :],
                            start=True, stop=False, skip_group_check=True,
                        )
                        nc.tensor.matmul(
                            zat[hp : hp + D, 1, P:N],
                            ones64,
                            est[:, sub, 2, :],
                            start=False, stop=True, skip_group_check=True,
                        )
                    for sub in range(2):
                        h = hp_i * 2 + sub
                        hp = D * sub
                        nc.tensor.matmul(
                            zat[hp : hp + D, 0, :],
                            v_sb[:, 0, h * D : (h + 1) * D],
                            est[:, sub, 0:2, :],
                            start=True, stop=False, skip_group_check=True,
                        )
                        nc.tensor.matmul(
                            zat[hp : hp + D, 0, P:N],
                            v_sb[:, 1, h * D : (h + 1) * D],
                            est[:, sub, 2, :],
                            start=False, stop=True, skip_group_check=True,
                        )
                    # 1/Z = exp(-ln Z): both funcs share one ACT table, so
                    # this is two fast fixed-function ops (no InstReciprocal,
                    # which runs at ~5.5 cycles/elem on DVE).
                    lz = heads.tile([P, N], F32, tag="lz")
                    nc.scalar.activation(lz, zat[:, 1, :], AF.Ln)
                    rz = heads.tile([P, N], F32, tag="rz")
                    nc.scalar.activation(rz, lz, AF.Exp, scale=-1.0)
                    nc.vector.tensor_mul(at_sbs[e][:, hp_i, :], zat[:, 0, :], rz)

            # ---- output projection: y [N, C] = attnT^T @ Wo ----
            for e in range(2):
                i = pi * 2 + e
                for t in range(TB):
                    py = ps_mm.tile([P, C], F32, tag="mm")
                    for k in range(CB):
                        nc.tensor.matmul(
                            py,
                            at_sbs[e][:, k, t * P : (t + 1) * P],
                            wo_sb[:, k, :],
                            start=(k == 0),
                            stop=(k == CB - 1),
                        )
                    y_sb = io.tile([P, C], F32, tag="y")
                    nc.vector.tensor_copy(y_sb, py)
                    nc.sync.dma_start(
                        y[i].rearrange("(t p) c -> p t c", p=P)[:, t, :], y_sb
                    )

    return nc


_NC_CACHE: dict = {}


def _build(nb: int = NB) -> bass.Bass:
    key = nb
    if key not in _NC_CACHE:
        nc = bacc.Bacc()
        _emit(nc, nb)
        nc.finalize()
        _NC_CACHE[key] = nc
    return _NC_CACHE[key]


def _run(inputs: dict, nb: int = NB, trace: bool = False):
    """Returns (y_full [8*nb, N, C], BassKernelResults)."""
    from concourse.bass_utils import run_bass_kernel_spmd

    import ml_dtypes

    bf16 = ml_dtypes.bfloat16
    x = np.asarray(inputs["x"], np.float32)[: NCORES * nb]
    xt = np.ascontiguousarray(x.transpose(0, 2, 1)).astype(bf16)
    Wq = np.ascontiguousarray(np.asarray(inputs["Wq"], np.float32).astype(bf16))
    Wk = np.ascontiguousarray(np.asarray(inputs["Wk"], np.float32).astype(bf16))
    Wv = np.ascontiguousarray(np.asarray(inputs["Wv"], np.float32).astype(bf16))
    Wo = np.ascontiguousarray(np.asarray(inputs["Wo"], np.float32).astype(bf16))
    bq = np.ascontiguousarray(np.asarray(inputs["bq"], np.float32))
    bk = np.ascontiguousarray(np.asarray(inputs["bk"], np.float32))
    bv = np.asarray(inputs["bv"], np.float32)
    bo = np.asarray(inputs["bo"], np.float32)

    nc = _build(nb)
    in_maps = [
        {
            "xt": np.ascontiguousarray(xt[c * nb : (c + 1) * nb]),
            "Wq": Wq,
            "Wk": Wk,
            "Wv": Wv,
            "Wo": Wo,
            "bq": bq,
            "bk": bk,
        }
        for c in range(NCORES)
    ]
    res = run_bass_kernel_spmd(nc, in_maps, list(range(NCORES)), trace=trace)
    y = np.concatenate([r["y"] for r in res.results], axis=0)
    # host-side fold of bv/bo (exact: softmax rows sum to 1)
    y = y + (bv @ np.asarray(inputs["Wo"], np.float32) + bo)
    return y, res


def kernel(**inputs) -> np.ndarray:
    y, _ = _run(inputs, nb=NB, trace=False)
    return y.astype(np.float32)


# revision 9
# speedup vs baseline: 1.6163x; 1.6163x over previous
"""Trainium2 Bass kernel for a causal multi-head attention block.

Problem: y = MHA(x), B=256, N=256 (seq), C=512, H=8 heads, d=64
  Q = x@Wq + bq ; K = x@Wk + bk ; V = x@Wv + bv   (per-head split)
  S = Q K^T ; scaled = (S + causal_mask*-1e5)/sqrt(d) ; P = softmax(scaled)
  y = (P V merged) @ Wo + bo

Sharding: pure data-parallel over batch B across 8 NeuronCores (32 batch
elements per core); weights replicated; no collectives.

Host-side marshalling inside kernel():
 - x is pre-transposed to xT [b, C, N] and cast to bf16.
 - weights pre-cast to bf16.
 - bv/bo folded on host: softmax rows sum to 1, so V's bias contributes
   attn@(1 bv^T) = bv exactly, hence y += bv@Wo + bo after gathering.

Device pipeline (transpose-free attention):
 - QT/KT [C, 2N] = W^T @ xT computed per batch-element pair; bq/bk fused
   into the PSUM->SBUF evacuation (ACT Identity + per-partition bias).
 - V [N, C] = x @ Wv per element (DVE evac).
 - Scores are computed TRANSPOSED: ST[k, q] = K_h^T q stored with keys on
   partitions, by swapping the matmul operands (lhsT=K_h, rhs=Q_h). Only
   the 3 causally-live 128x128 blocks are computed:
     blk0 = (k0, q0), blk1 = (k0, q1), blk2 = (k1, q1).
   Two heads run concurrently in disjoint 64-row PE groups.
 - exp via one ACT op per head (scale=1/8, PSUM->SBUF bf16, no max-sub:
   scores are O(1)). Causal masking = multiply the two diagonal blocks by
   a 0/1 lower-triangular bf16 constant on DVE (masked reference lanes
   underflow exp->0 exactly, so 0 matches bit-for-bit).
 - Z[q] (softmax denominators) via ones-matmul column sums on the PE:
   zpair[64h..64h+64, q] = ones64^T @ expST_h, head pairs col-packed.
 - attnT_raw [d, q] = V_h^T @ expST_h straight into the [C, N] layout
   (col-packed pairs). Normalization happens on the small attnT tile:
   at_sb = at_ps * reciprocal(zpair) on DVE (1/Z broadcast over d rows
   comes for free since zpair rows are all identical per head).
 - y [N, C] = attnT^T @ Wo, DVE evac, DMA out.

No PE transposes anywhere -> tensor engine stays HAM-warm and does only
real matmul work.
"""

import os
import sys

sys.path.insert(0, "/opt/trn_rl_repo")

import numpy as np

import concourse.bass as bass
import concourse.mybir as mybir
import concourse.tile as tile
from concourse import bacc

B, N, C, H, D = 256, 256, 512, 8, 64
NCORES = 8
NB = B // NCORES  # batch elements per core
P = 128
F32 = mybir.dt.float32

AF = mybir.ActivationFunctionType

MM_DT = mybir.dt.bfloat16
PV_DT = mybir.dt.bfloat16


def _emit(nc: bass.Bass, nb: int):
    xt_in = nc.dram_tensor("xt", [nb, C, N], MM_DT, kind="ExternalInput")
    Wq = nc.dram_tensor("Wq", [C, C], MM_DT, kind="ExternalInput")
    Wk = nc.dram_tensor("Wk", [C, C], MM_DT, kind="ExternalInput")
    Wv = nc.dram_tensor("Wv", [C, C], MM_DT, kind="ExternalInput")
    Wo = nc.dram_tensor("Wo", [C, C], MM_DT, kind="ExternalInput")
    bq = nc.dram_tensor("bq", [C], F32, kind="ExternalInput")
    bk = nc.dram_tensor("bk", [C], F32, kind="ExternalInput")
    y = nc.dram_tensor("y", [nb, N, C], F32, kind="ExternalOutput")

    CB = C // P  # 4 column blocks of 128
    TB = N // P  # 2 row blocks of 128

    with (
        tile.TileContext(nc) as tc,
        tc.tile_pool(name="consts", bufs=1) as consts,
        tc.tile_pool(name="io", bufs=3) as io,
        tc.tile_pool(name="work", bufs=3) as work,
        tc.tile_pool(name="heads", bufs=int(os.environ.get("HB", "4"))) as heads,
        tc.tile_pool(name="ps_mm", bufs=int(os.environ.get("PS_MM", "2")), space="PSUM") as ps_mm,
        tc.tile_pool(name="ps_sc", bufs=int(os.environ.get("PS_SC", "2")), space="PSUM") as ps_sc,
        tc.tile_pool(name="ps_zat", bufs=int(os.environ.get("PS_ZAT", "2")), space="PSUM") as ps_zat,
    ):
        # ---- constants ----
        # keep-mask for the two causal diagonal blocks of both heads of a
        # pair: 1 where k <= q (p >= f), else 0.  Layout [P, sub, blk, P].
        mk4 = consts.tile([P, 2, 2, P], PV_DT)
        nc.gpsimd.memset(mk4, 1.0)
        for i in range(2):
            for j in range(2):
                nc.gpsimd.affine_select(
                    out=mk4[:, i, j, :],
                    in_=mk4[:, i, j, :],
                    compare_op=mybir.AluOpType.is_ge,
                    fill=0.0,
                    base=0,
                    pattern=[[1, P]],
                    channel_multiplier=-1,
                )
        ones64 = consts.tile([P, D], PV_DT)
        nc.gpsimd.memset(ones64, 1.0)

        wq_sb = consts.tile([P, CB, C], MM_DT)
        nc.sync.dma_start(wq_sb, Wq.rearrange("(k p) m -> p k m", p=P))
        wk_sb = consts.tile([P, CB, C], MM_DT)
        nc.sync.dma_start(wk_sb, Wk.rearrange("(k p) m -> p k m", p=P))
        wv_sb = consts.tile([P, CB, C], MM_DT)
        nc.sync.dma_start(wv_sb, Wv.rearrange("(k p) m -> p k m", p=P))
        wo_sb = consts.tile([P, CB, C], MM_DT)
        nc.sync.dma_start(wo_sb, Wo.rearrange("(k p) m -> p k m", p=P))
        bq_sb = consts.tile([P, CB], F32)
        nc.sync.dma_start(bq_sb, bq.rearrange("(m p) -> p m", p=P))
        bk_sb = consts.tile([P, CB], F32)
        nc.sync.dma_start(bk_sb, bk.rearrange("(m p) -> p m", p=P))

        for pi in range(nb // 2):
            # ---- load pair of batch elems (already transposed on host) ----
            xT = work.tile([P, CB, 2 * N], MM_DT, tag="xT")
            for e in range(2):
                i = pi * 2 + e
                nc.sync.dma_start(
                    xT[:, :, e * N : (e + 1) * N],
                    xt_in[i].rearrange("(cb p) n -> p cb n", p=P),
                )

            # ---- paired projections: QT/KT [C, 2N] = W^T @ xT ----
            qt = work.tile([P, CB, 2 * N], MM_DT, tag="qt")
            kt = work.tile([P, CB, 2 * N], MM_DT, tag="kt")
            for mb in range(CB):
                pq = ps_mm.tile([P, 2 * N], F32, tag="mm")
                for k in range(CB):
                    nc.tensor.matmul(
                        pq,
                        wq_sb[:, k, mb * P : (mb + 1) * P],
                        xT[:, k, :],
                        start=(k == 0),
                        stop=(k == CB - 1),
                    )
                nc.vector.tensor_scalar_add(
                    qt[:, mb, :], pq, bq_sb[:, mb : mb + 1]
                )
                pk = ps_mm.tile([P, 2 * N], F32, tag="mm")
                for k in range(CB):
                    nc.tensor.matmul(
                        pk,
                        wk_sb[:, k, mb * P : (mb + 1) * P],
                        xT[:, k, :],
                        start=(k == 0),
                        stop=(k == CB - 1),
                    )
                nc.vector.tensor_scalar_add(
                    kt[:, mb, :], pk, bk_sb[:, mb : mb + 1]
                )

            # ---- V [N, C] = x @ Wv for both elements of the pair ----
            v_sbs = []
            for e in range(2):
                eo = e * N
                v_sb = work.tile([P, TB, C], PV_DT, tag=f"v{e}")
                for t in range(TB):
                    pv = ps_mm.tile([P, C], F32, tag="mm")
                    for k in range(CB):
                        nc.tensor.matmul(
                            pv,
                            xT[:, k, eo + t * P : eo + (t + 1) * P],
                            wv_sb[:, k, :],
                            start=(k == 0),
                            stop=(k == CB - 1),
                        )
                    nc.vector.tensor_copy(v_sb[:, t, :], pv)
                v_sbs.append(v_sb)

            # ---- attention, transposed scores; elements interleaved so
            # the PE never waits on a single element's softmax tail ----
            at_sbs = [
                work.tile([P, CB, N], MM_DT, tag=f"at{e}", name=f"at_sb{e}")
                for e in range(2)
            ]
            for hp_i in range(H // 2):
                for e in range(2):
                    eo = e * N
                    v_sb = v_sbs[e]
                    # ST blocks for both heads of the pair.
                    # s_pair[:, sub, 0:2, :] = K_h(k0)^T Q_h  (q0 | q1)
                    # s_pair[:, sub, 2, :]   = K_h(k1)^T Q_h  (q1)
                    # sub-tiles sit in separate PSUM banks (4 blks * 512B).
                    s_pair = ps_sc.tile([P, 2, 4, P], F32, tag="sc")
                    for sub in range(2):
                        hp = D * sub
                        qh = qt[hp : hp + D, hp_i, eo : eo + N]
                        kh = kt[hp : hp + D, hp_i, eo : eo + N]
                        nc.tensor.matmul(
                            s_pair[:, sub, 0:2, :], kh[:, 0:P], qh,
                            start=True, stop=True, skip_group_check=True,
                        )
                        nc.tensor.matmul(
                            s_pair[:, sub, 2, :], kh[:, P:N], qh[:, P:N],
                            start=True, stop=True, skip_group_check=True,
                        )
                    # exp((S+mask)/sqrt(d)) without mask: masked lanes are
                    # zeroed right after (reference underflows to 0 too).
                    est = heads.tile([P, 2, 3, P], PV_DT, tag="est")
                    nc.scalar.activation(
                        est, s_pair[:, 0:2, 0:3, :], AF.Exp, scale=0.125
                    )
                    nc.gpsimd.tensor_mul(
                        est[:, 0:2, 0:3:2, :], est[:, 0:2, 0:3:2, :], mk4
                    )
                    # Z column sums (ones-matmul) + attnT_raw, col-packed,
                    # sharing one PSUM bank ([:, 0, :] = attnT, [:, 1, :] = Z).
                    zat = ps_zat.tile([P, 2, N], F32, tag="zat")
                    for sub in range(2):
                        hp = D * sub
                        nc.tensor.matmul(
                            zat[hp : hp + D, 1, :],
                            ones64,
                            est[:, sub, 0:2, :],
                            start=True, stop=False, skip_group_check=True,
                        )
                        nc.tensor.matmul(
                            zat[hp : hp + D, 1, P:N],
                            ones64,
                            est[:, sub, 2, :],
                            start=False, stop=True, skip_group_check=True,
                        )
                    for sub in range(2):
                        h = hp_i * 2 + sub
                        hp = D * sub
                        nc.tensor.matmul(
                            zat[hp : hp + D, 0, :],
                            v_sb[:, 0, h * D : (h + 1) * D],
                            est[:, sub, 0:2, :],
                            start=True, stop=False, skip_group_check=True,
                        )
                        nc.tensor.matmul(
                            zat[hp : hp + D, 0, P:N],
                            v_sb[:, 1, h * D : (h + 1) * D],
                            est[:, sub, 2, :],
                            start=False, stop=True, skip_group_check=True,
                        )
                    # 1/Z = exp(-ln Z): both funcs share one ACT table, so
                    # this is two fast fixed-function ops (no InstReciprocal,
                    # which runs at ~5.5 cycles/elem on DVE).
                    lz = heads.tile([P, N], F32, tag="lz")
                    nc.scalar.activation(lz, zat[:, 1, :], AF.Ln)
                    rz = heads.tile([P, N], F32, tag="rz")
                    nc.scalar.activation(rz, lz, AF.Exp, scale=-1.0)
                    nc.vector.tensor_mul(at_sbs[e][:, hp_i, :], zat[:, 0, :], rz)

            # ---- output projection: y [N, C] = attnT^T @ Wo ----
            for e in range(2):
                i = pi * 2 + e
                for t in range(TB):
                    py = ps_mm.tile([P, C], F32, tag="mm")
                    for k in range(CB):
                        nc.tensor.matmul(
                            py,
                            at_sbs[e][:, k, t * P : (t + 1) * P],
                            wo_sb[:, k, :],
                            start=(k == 0),
                            stop=(k == CB - 1),
                        )
                    y_sb = io.tile([P, C], F32, tag="y")
                    nc.vector.tensor_copy(y_sb, py)
                    nc.sync.dma_start(
                        y[i].rearrange("(t p) c -> p t c", p=P)[:, t, :], y_sb
                    )

    return nc


_NC_CACHE: dict = {}


class _PinnedActBacc(bacc.Bacc):
    """Bacc that pins every activation to the one act-func table holding
    both exp and ln ("natural_log_exp_and_others"), so the 1/Z = exp(-ln Z)
    path doesn't thrash 1283ns ACT_TABLE_LOADs between exp and ln tables.
    Table ids stay indices into the unmodified act_info.json, so execution
    is unchanged -- this only steers the compile-time table choice."""

    def insert_act_table_loads(self):
        import bass_rust as _bass_rust
        from concourse.hw_specs import get_activation_tables

        has_activation = any(
            isinstance(i, mybir.InstActivation)
            for b in self.main_func.blocks
            for i in b.instructions
        )
        if not has_activation:
            return
        pin = {AF.Exp, AF.Ln, AF.Identity, AF.Copy}
        tables = []
        for name, funcs in get_activation_tables(self.m.arch).items():
            if name != "natural_log_exp_and_others":
                funcs = funcs - pin
            tables.append((name, funcs))
        _bass_rust.insert_act_table_loads(self, tables)


def _build(nb: int = NB) -> bass.Bass:
    key = nb
    if key not in _NC_CACHE:
        nc = _PinnedActBacc()
        _emit(nc, nb)
        nc.finalize()
        _NC_CACHE[key] = nc
    return _NC_CACHE[key]


def _run(inputs: dict, nb: int = NB, trace: bool = False):
    """Returns (y_full [8*nb, N, C], BassKernelResults)."""
    from concourse.bass_utils import run_bass_kernel_spmd

    import ml_dtypes

    bf16 = ml_dtypes.bfloat16
    x = np.asarray(inputs["x"], np.float32)[: NCORES * nb]
    xt = np.ascontiguousarray(x.transpose(0, 2, 1)).astype(bf16)
    Wq = np.ascontiguousarray(np.asarray(inputs["Wq"], np.float32).astype(bf16))
    Wk = np.ascontiguousarray(np.asarray(inputs["Wk"], np.float32).astype(bf16))
    Wv = np.ascontiguousarray(np.asarray(inputs["Wv"], np.float32).astype(bf16))
    Wo = np.ascontiguousarray(np.asarray(inputs["Wo"], np.float32).astype(bf16))
    bq = np.ascontiguousarray(np.asarray(inputs["bq"], np.float32))
    bk = np.ascontiguousarray(np.asarray(inputs["bk"], np.float32))
    bv = np.asarray(inputs["bv"], np.float32)
    bo = np.asarray(inputs["bo"], np.float32)

    nc = _build(nb)
    in_maps = [
        {
            "xt": np.ascontiguousarray(xt[c * nb : (c + 1) * nb]),
            "Wq": Wq,
            "Wk": Wk,
            "Wv": Wv,
            "Wo": Wo,
            "bq": bq,
            "bk": bk,
        }
        for c in range(NCORES)
    ]
    res = run_bass_kernel_spmd(nc, in_maps, list(range(NCORES)), trace=trace)
    y = np.concatenate([r["y"] for r in res.results], axis=0)
    # host-side fold of bv/bo (exact: softmax rows sum to 1)
    y = y + (bv @ np.asarray(inputs["Wo"], np.float32) + bo)
    return y, res


def kernel(**inputs) -> np.ndarray:
    y, _ = _run(inputs, nb=NB, trace=False)
    return y.astype(np.float32)


# revision 10
# speedup vs baseline: 1.9011x; 1.1762x over previous
"""Trainium2 Bass kernel for a causal multi-head attention block.

Problem: y = MHA(x), B=256, N=256 (seq), C=512, H=8 heads, d=64
  Q = x@Wq + bq ; K = x@Wk + bk ; V = x@Wv + bv   (per-head split)
  S = Q K^T ; scaled = (S + causal_mask*-1e5)/sqrt(d) ; P = softmax(scaled)
  y = (P V merged) @ Wo + bo

Sharding: pure data-parallel over batch B across 8 NeuronCores (32 batch
elements per core); weights replicated; no collectives.

Host-side marshalling inside kernel():
 - x is pre-transposed to xT [b, C, N] and cast to bf16.
 - weights pre-cast to bf16.
 - bv/bo folded on host: softmax rows sum to 1, so V's bias contributes
   attn@(1 bv^T) = bv exactly, hence y += bv@Wo + bo after gathering.

Device pipeline (transpose-free attention):
 - QT/KT [C, 2N] = W^T @ xT computed per batch-element pair; bq/bk fused
   into the PSUM->SBUF evacuation (ACT Identity + per-partition bias).
 - V [N, C] = x @ Wv per element (DVE evac).
 - Scores are computed TRANSPOSED: ST[k, q] = K_h^T q stored with keys on
   partitions, by swapping the matmul operands (lhsT=K_h, rhs=Q_h). Only
   the 3 causally-live 128x128 blocks are computed:
     blk0 = (k0, q0), blk1 = (k0, q1), blk2 = (k1, q1).
   Two heads run concurrently in disjoint 64-row PE groups.
 - exp via one ACT op per head (scale=1/8, PSUM->SBUF bf16, no max-sub:
   scores are O(1)). Causal masking = multiply the two diagonal blocks by
   a 0/1 lower-triangular bf16 constant on DVE (masked reference lanes
   underflow exp->0 exactly, so 0 matches bit-for-bit).
 - Z[q] (softmax denominators) via ones-matmul column sums on the PE:
   zpair[64h..64h+64, q] = ones64^T @ expST_h, head pairs col-packed.
 - attnT_raw [d, q] = V_h^T @ expST_h straight into the [C, N] layout
   (col-packed pairs). Normalization happens on the small attnT tile:
   at_sb = at_ps * reciprocal(zpair) on DVE (1/Z broadcast over d rows
   comes for free since zpair rows are all identical per head).
 - y [N, C] = attnT^T @ Wo, DVE evac, DMA out.

No PE transposes anywhere -> tensor engine stays HAM-warm and does only
real matmul work.
"""

import os
import sys

sys.path.insert(0, "/opt/trn_rl_repo")

import numpy as np

import concourse.bass as bass
import concourse.mybir as mybir
import concourse.tile as tile
from concourse import bacc

B, N, C, H, D = 256, 256, 512, 8, 64
NCORES = 8
NB = B // NCORES  # batch elements per core
P = 128
F32 = mybir.dt.float32

AF = mybir.ActivationFunctionType

MM_DT = mybir.dt.bfloat16
PV_DT = mybir.dt.bfloat16


def _emit(nc: bass.Bass, nb: int):
    xt_in = nc.dram_tensor("xt", [nb, C, N], MM_DT, kind="ExternalInput")
    Wq = nc.dram_tensor("Wq", [C, C], MM_DT, kind="ExternalInput")
    Wk = nc.dram_tensor("Wk", [C, C], MM_DT, kind="ExternalInput")
    Wv = nc.dram_tensor("Wv", [C, C], MM_DT, kind="ExternalInput")
    Wo = nc.dram_tensor("Wo", [C, C], MM_DT, kind="ExternalInput")
    bq = nc.dram_tensor("bq", [C], F32, kind="ExternalInput")
    bk = nc.dram_tensor("bk", [C], F32, kind="ExternalInput")
    y = nc.dram_tensor("y", [nb, N, C], F32, kind="ExternalOutput")

    CB = C // P  # 4 column blocks of 128
    TB = N // P  # 2 row blocks of 128

    with (
        tile.TileContext(nc) as tc,
        tc.tile_pool(name="consts", bufs=1) as consts,
        tc.tile_pool(name="io", bufs=3) as io,
        tc.tile_pool(name="work", bufs=3) as work,
        tc.tile_pool(name="heads", bufs=int(os.environ.get("HB", "4"))) as heads,
        tc.tile_pool(name="ps_mm", bufs=int(os.environ.get("PS_MM", "2")), space="PSUM") as ps_mm,
        tc.tile_pool(name="ps_sc", bufs=int(os.environ.get("PS_SC", "2")), space="PSUM") as ps_sc,
        tc.tile_pool(name="ps_zat", bufs=int(os.environ.get("PS_ZAT", "2")), space="PSUM") as ps_zat,
    ):
        # ---- constants ----
        # keep-mask for the two causal diagonal blocks of both heads of a
        # pair: 1 where k <= q (p >= f), else 0.  Layout [P, sub, blk, P].
        mk4 = consts.tile([P, 2, 2, P], PV_DT)
        nc.gpsimd.memset(mk4, 1.0)
        for i in range(2):
            for j in range(2):
                nc.gpsimd.affine_select(
                    out=mk4[:, i, j, :],
                    in_=mk4[:, i, j, :],
                    compare_op=mybir.AluOpType.is_ge,
                    fill=0.0,
                    base=0,
                    pattern=[[1, P]],
                    channel_multiplier=-1,
                )
        ones64 = consts.tile([P, D], PV_DT)
        nc.gpsimd.memset(ones64, 1.0)

        wq_sb = consts.tile([P, CB, C], MM_DT)
        nc.sync.dma_start(wq_sb, Wq.rearrange("(k p) m -> p k m", p=P))
        wk_sb = consts.tile([P, CB, C], MM_DT)
        nc.sync.dma_start(wk_sb, Wk.rearrange("(k p) m -> p k m", p=P))
        wv_sb = consts.tile([P, CB, C], MM_DT)
        nc.sync.dma_start(wv_sb, Wv.rearrange("(k p) m -> p k m", p=P))
        wo_sb = consts.tile([P, CB, C], MM_DT)
        nc.sync.dma_start(wo_sb, Wo.rearrange("(k p) m -> p k m", p=P))
        bq_sb = consts.tile([P, CB], F32)
        nc.sync.dma_start(bq_sb, bq.rearrange("(m p) -> p m", p=P))
        bk_sb = consts.tile([P, CB], F32)
        nc.sync.dma_start(bk_sb, bk.rearrange("(m p) -> p m", p=P))

        # ------------------------------------------------------------------
        # Software pipeline: each batch-element pair's DENSE work (proj/V,
        # big N=512 matmuls) is emitted interleaved with the PREVIOUS pair's
        # SPARSE work (attention: small matmuls gated on softmax round
        # trips).  This keeps PE activity dense everywhere so the HAM clock
        # gate stays at full rate.
        # ------------------------------------------------------------------

        def make_dense(pi):
            """Thunks for proj+V of pair pi. Returns (thunks, state)."""
            state = {}

            def load_x():
                xT = work.tile([P, CB, 2 * N], MM_DT, tag="xT", name="xT")
                for e in range(2):
                    i = pi * 2 + e
                    nc.sync.dma_start(
                        xT[:, :, e * N : (e + 1) * N],
                        xt_in[i].rearrange("(cb p) n -> p cb n", p=P),
                    )
                state["xT"] = xT
                state["qt"] = work.tile([P, CB, 2 * N], MM_DT, tag="qt", name="qt")
                state["kt"] = work.tile([P, CB, 2 * N], MM_DT, tag="kt", name="kt")
                state["v"] = [
                    work.tile([P, TB, C], PV_DT, tag=f"v{e}", name=f"v{e}")
                    for e in range(2)
                ]

            def proj(mb, which):
                xT = state["xT"]
                w_sb, b_sb, out = (
                    (wq_sb, bq_sb, state["qt"])
                    if which == "q"
                    else (wk_sb, bk_sb, state["kt"])
                )
                pq = ps_mm.tile([P, 2 * N], F32, tag="mm", name="pq")
                for k in range(CB):
                    nc.tensor.matmul(
                        pq,
                        w_sb[:, k, mb * P : (mb + 1) * P],
                        xT[:, k, :],
                        start=(k == 0),
                        stop=(k == CB - 1),
                    )
                nc.vector.tensor_scalar_add(
                    out[:, mb, :], pq, b_sb[:, mb : mb + 1]
                )

            def vproj(e, t):
                xT = state["xT"]
                eo = e * N
                pv = ps_mm.tile([P, C], F32, tag="mm", name="pv")
                for k in range(CB):
                    nc.tensor.matmul(
                        pv,
                        xT[:, k, eo + t * P : eo + (t + 1) * P],
                        wv_sb[:, k, :],
                        start=(k == 0),
                        stop=(k == CB - 1),
                    )
                nc.vector.tensor_copy(state["v"][e][:, t, :], pv)

            thunks = [load_x]
            for mb in range(CB):
                thunks.append(lambda mb=mb: proj(mb, "q"))
                thunks.append(lambda mb=mb: proj(mb, "k"))
            for e in range(2):
                for t in range(TB):
                    thunks.append(lambda e=e, t=t: vproj(e, t))
            return thunks, state

        def make_sparse(pi, state):
            """Thunks for attention + out-proj of pair pi."""
            qt, kt = state["qt"], state["kt"]
            at_sbs = [
                work.tile([P, CB, N], MM_DT, tag=f"at{e}", name=f"at_sb{e}")
                for e in range(2)
            ]

            def attn(hp_i, e):
                eo = e * N
                v_sb = state["v"][e]
                # ST blocks for both heads of the pair:
                # s_pair[:, sub, 0:2, :] = K_h(k0)^T Q_h  (q0 | q1)
                # s_pair[:, sub, 2, :]   = K_h(k1)^T Q_h  (q1)
                s_pair = ps_sc.tile([P, 2, 4, P], F32, tag="sc", name="s_pair")
                for sub in range(2):
                    hp = D * sub
                    qh = qt[hp : hp + D, hp_i, eo : eo + N]
                    kh = kt[hp : hp + D, hp_i, eo : eo + N]
                    nc.tensor.matmul(
                        s_pair[:, sub, 0:2, :], kh[:, 0:P], qh,
                        start=True, stop=True, skip_group_check=True,
                    )
                    nc.tensor.matmul(
                        s_pair[:, sub, 2, :], kh[:, P:N], qh[:, P:N],
                        start=True, stop=True, skip_group_check=True,
                    )
                # exp((S+mask)/sqrt(d)) without mask: masked lanes are
                # zeroed right after (reference underflows to 0 too).
                est = heads.tile([P, 2, 3, P], PV_DT, tag="est", name="est")
                nc.scalar.activation(
                    est, s_pair[:, 0:2, 0:3, :], AF.Exp, scale=0.125
                )
                nc.gpsimd.tensor_mul(
                    est[:, 0:2, 0:3:2, :], est[:, 0:2, 0:3:2, :], mk4
                )
                # Z column sums (ones-matmul) + attnT_raw, col-packed,
                # sharing one PSUM bank ([:, 0, :] = attnT, [:, 1, :] = Z).
                zat = ps_zat.tile([P, 2, N], F32, tag="zat", name="zat")
                for sub in range(2):
                    hp = D * sub
                    nc.tensor.matmul(
                        zat[hp : hp + D, 1, :],
                        ones64,
                        est[:, sub, 0:2, :],
                        start=True, stop=False, skip_group_check=True,
                    )
                    nc.tensor.matmul(
                        zat[hp : hp + D, 1, P:N],
                        ones64,
                        est[:, sub, 2, :],
                        start=False, stop=True, skip_group_check=True,
                    )
                for sub in range(2):
                    h = hp_i * 2 + sub
                    hp = D * sub
                    nc.tensor.matmul(
                        zat[hp : hp + D, 0, :],
                        v_sb[:, 0, h * D : (h + 1) * D],
                        est[:, sub, 0:2, :],
                        start=True, stop=False, skip_group_check=True,
                    )
                    nc.tensor.matmul(
                        zat[hp : hp + D, 0, P:N],
                        v_sb[:, 1, h * D : (h + 1) * D],
                        est[:, sub, 2, :],
                        start=False, stop=True, skip_group_check=True,
                    )
                # 1/Z = exp(-ln Z): both funcs live in one ACT table (the
                # build pins it), so no InstReciprocal and no table thrash.
                lz = heads.tile([P, N], F32, tag="lz", name="lz")
                nc.scalar.activation(lz, zat[:, 1, :], AF.Ln)
                rz = heads.tile([P, N], F32, tag="rz", name="rz")
                nc.scalar.activation(rz, lz, AF.Exp, scale=-1.0)
                nc.vector.tensor_mul(at_sbs[e][:, hp_i, :], zat[:, 0, :], rz)

            def outproj(e, t):
                i = pi * 2 + e
                py = ps_mm.tile([P, C], F32, tag="mm", name="py")
                for k in range(CB):
                    nc.tensor.matmul(
                        py,
                        at_sbs[e][:, k, t * P : (t + 1) * P],
                        wo_sb[:, k, :],
                        start=(k == 0),
                        stop=(k == CB - 1),
                    )
                y_sb = io.tile([P, C], F32, tag="y", name="y_sb")
                nc.vector.tensor_copy(y_sb, py)
                nc.sync.dma_start(
                    y[i].rearrange("(t p) c -> p t c", p=P)[:, t, :], y_sb
                )

            thunks = []
            for hp_i in range(H // 2):
                for e in range(2):
                    thunks.append(lambda hp_i=hp_i, e=e: attn(hp_i, e))
            for e in range(2):
                for t in range(TB):
                    thunks.append(lambda e=e, t=t: outproj(e, t))
            return thunks

        prev_sparse = []
        for pi in range(nb // 2):
            dense, state = make_dense(pi)
            # interleave: dense thunks of pair pi with sparse of pair pi-1
            n = max(len(dense), len(prev_sparse))
            for j in range(n):
                if j < len(dense):
                    dense[j]()
                if j < len(prev_sparse):
                    prev_sparse[j]()
            prev_sparse = make_sparse(pi, state)
        for t in prev_sparse:
            t()

    return nc


_NC_CACHE: dict = {}


class _PinnedActBacc(bacc.Bacc):
    """Bacc that pins every activation to the one act-func table holding
    both exp and ln ("natural_log_exp_and_others"), so the 1/Z = exp(-ln Z)
    path doesn't thrash 1283ns ACT_TABLE_LOADs between exp and ln tables.
    Table ids stay indices into the unmodified act_info.json, so execution
    is unchanged -- this only steers the compile-time table choice."""

    def insert_act_table_loads(self):
        import bass_rust as _bass_rust
        from concourse.hw_specs import get_activation_tables

        has_activation = any(
            isinstance(i, mybir.InstActivation)
            for b in self.main_func.blocks
            for i in b.instructions
        )
        if not has_activation:
            return
        pin = {AF.Exp, AF.Ln, AF.Identity, AF.Copy}
        tables = []
        for name, funcs in get_activation_tables(self.m.arch).items():
            if name != "natural_log_exp_and_others":
                funcs = funcs - pin
            tables.append((name, funcs))
        _bass_rust.insert_act_table_loads(self, tables)


def _build(nb: int = NB) -> bass.Bass:
    key = nb
    if key not in _NC_CACHE:
        nc = _PinnedActBacc()
        _emit(nc, nb)
        nc.finalize()
        _NC_CACHE[key] = nc
    return _NC_CACHE[key]


def _run(inputs: dict, nb: int = NB, trace: bool = False):
    """Returns (y_full [8*nb, N, C], BassKernelResults)."""
    from concourse.bass_utils import run_bass_kernel_spmd

    import ml_dtypes

    bf16 = ml_dtypes.bfloat16
    x = np.asarray(inputs["x"], np.float32)[: NCORES * nb]
    xt = np.ascontiguousarray(x.transpose(0, 2, 1)).astype(bf16)
    Wq = np.ascontiguousarray(np.asarray(inputs["Wq"], np.float32).astype(bf16))
    Wk = np.ascontiguousarray(np.asarray(inputs["Wk"], np.float32).astype(bf16))
    Wv = np.ascontiguousarray(np.asarray(inputs["Wv"], np.float32).astype(bf16))
    Wo = np.ascontiguousarray(np.asarray(inputs["Wo"], np.float32).astype(bf16))
    bq = np.ascontiguousarray(np.asarray(inputs["bq"], np.float32))
    bk = np.ascontiguousarray(np.asarray(inputs["bk"], np.float32))
    bv = np.asarray(inputs["bv"], np.float32)
    bo = np.asarray(inputs["bo"], np.float32)

    nc = _build(nb)
    in_maps = [
        {
            "xt": np.ascontiguousarray(xt[c * nb : (c + 1) * nb]),
            "Wq": Wq,
            "Wk": Wk,
            "Wv": Wv,
            "Wo": Wo,
            "bq": bq,
            "bk": bk,
        }
        for c in range(NCORES)
    ]
    res = run_bass_kernel_spmd(nc, in_maps, list(range(NCORES)), trace=trace)
    y = np.concatenate([r["y"] for r in res.results], axis=0)
    # host-side fold of bv/bo (exact: softmax rows sum to 1)
    y = y + (bv @ np.asarray(inputs["Wo"], np.float32) + bo)
    return y, res


def kernel(**inputs) -> np.ndarray:
    y, _ = _run(inputs, nb=NB, trace=False)
    return y.astype(np.float32)


# revision 12
# speedup vs baseline: 1.9047x; 1.0019x over previous
"""Trainium2 Bass kernel for a causal multi-head attention block.

Problem: y = MHA(x), B=256, N=256 (seq), C=512, H=8 heads, d=64
  Q = x@Wq + bq ; K = x@Wk + bk ; V = x@Wv + bv   (per-head split)
  S = Q K^T ; scaled = (S + causal_mask*-1e5)/sqrt(d) ; P = softmax(scaled)
  y = (P V merged) @ Wo + bo

Sharding: pure data-parallel over batch B across 8 NeuronCores (32 batch
elements per core); weights replicated; no collectives.

Host-side marshalling inside kernel():
 - x is pre-transposed to xT [b, C, N] and cast to bf16.
 - weights pre-cast to bf16.
 - bv/bo folded on host: softmax rows sum to 1, so V's bias contributes
   attn@(1 bv^T) = bv exactly, hence y += bv@Wo + bo after gathering.

Device pipeline (transpose-free attention):
 - QT/KT [C, 2N] = W^T @ xT computed per batch-element pair; bq/bk fused
   into the PSUM->SBUF evacuation (ACT Identity + per-partition bias).
 - V [N, C] = x @ Wv per element (DVE evac).
 - Scores are computed TRANSPOSED: ST[k, q] = K_h^T q stored with keys on
   partitions, by swapping the matmul operands (lhsT=K_h, rhs=Q_h). Only
   the 3 causally-live 128x128 blocks are computed:
     blk0 = (k0, q0), blk1 = (k0, q1), blk2 = (k1, q1).
   Two heads run concurrently in disjoint 64-row PE groups.
 - exp via one ACT op per head (scale=1/8, PSUM->SBUF bf16, no max-sub:
   scores are O(1)). Causal masking = multiply the two diagonal blocks by
   a 0/1 lower-triangular bf16 constant on DVE (masked reference lanes
   underflow exp->0 exactly, so 0 matches bit-for-bit).
 - Z[q] (softmax denominators) via ones-matmul column sums on the PE:
   zpair[64h..64h+64, q] = ones64^T @ expST_h, head pairs col-packed.
 - attnT_raw [d, q] = V_h^T @ expST_h straight into the [C, N] layout
   (col-packed pairs). Normalization happens on the small attnT tile:
   at_sb = at_ps * reciprocal(zpair) on DVE (1/Z broadcast over d rows
   comes for free since zpair rows are all identical per head).
 - y [N, C] = attnT^T @ Wo, DVE evac, DMA out.

No PE transposes anywhere -> tensor engine stays HAM-warm and does only
real matmul work.
"""

import os
import sys

sys.path.insert(0, "/opt/trn_rl_repo")

import numpy as np

import concourse.bass as bass
import concourse.mybir as mybir
import concourse.tile as tile
from concourse import bacc

B, N, C, H, D = 256, 256, 512, 8, 64
NCORES = 8
NB = B // NCORES  # batch elements per core
P = 128
F32 = mybir.dt.float32

AF = mybir.ActivationFunctionType

MM_DT = mybir.dt.bfloat16
PV_DT = mybir.dt.bfloat16


def _emit(nc: bass.Bass, nb: int):
    xt_in = nc.dram_tensor("xt", [nb, C, N], MM_DT, kind="ExternalInput")
    Wq = nc.dram_tensor("Wq", [C, C], MM_DT, kind="ExternalInput")
    Wk = nc.dram_tensor("Wk", [C, C], MM_DT, kind="ExternalInput")
    Wv = nc.dram_tensor("Wv", [C, C], MM_DT, kind="ExternalInput")
    Wo = nc.dram_tensor("Wo", [C, C], MM_DT, kind="ExternalInput")
    bq = nc.dram_tensor("bq", [C], F32, kind="ExternalInput")
    bk = nc.dram_tensor("bk", [C], F32, kind="ExternalInput")
    y = nc.dram_tensor("y", [nb, N, C], F32, kind="ExternalOutput")

    CB = C // P  # 4 column blocks of 128
    TB = N // P  # 2 row blocks of 128

    with (
        tile.TileContext(nc) as tc,
        tc.tile_pool(name="consts", bufs=1) as consts,
        tc.tile_pool(name="io", bufs=3) as io,
        tc.tile_pool(name="work", bufs=3) as work,
        tc.tile_pool(name="heads", bufs=int(os.environ.get("HB", "6"))) as heads,
        tc.tile_pool(name="ps_mm", bufs=int(os.environ.get("PS_MM", "3")), space="PSUM") as ps_mm,
        tc.tile_pool(name="ps_sc", bufs=int(os.environ.get("PS_SC", "3")), space="PSUM") as ps_sc,
        tc.tile_pool(name="ps_zat", bufs=int(os.environ.get("PS_ZAT", "2")), space="PSUM") as ps_zat,
    ):
        # ---- constants ----
        # keep-mask for the two causal diagonal blocks of both heads of a
        # pair: 1 where k <= q (p >= f), else 0.  Layout [P, sub, blk, P].
        mk4 = consts.tile([P, 2, 2, P], PV_DT)
        nc.gpsimd.memset(mk4, 1.0)
        for i in range(2):
            for j in range(2):
                nc.gpsimd.affine_select(
                    out=mk4[:, i, j, :],
                    in_=mk4[:, i, j, :],
                    compare_op=mybir.AluOpType.is_ge,
                    fill=0.0,
                    base=0,
                    pattern=[[1, P]],
                    channel_multiplier=-1,
                )
        ones64 = consts.tile([P, D], PV_DT)
        nc.gpsimd.memset(ones64, 1.0)

        wq_sb = consts.tile([P, CB, C], MM_DT)
        nc.sync.dma_start(wq_sb, Wq.rearrange("(k p) m -> p k m", p=P))
        wk_sb = consts.tile([P, CB, C], MM_DT)
        nc.sync.dma_start(wk_sb, Wk.rearrange("(k p) m -> p k m", p=P))
        wv_sb = consts.tile([P, CB, C], MM_DT)
        nc.sync.dma_start(wv_sb, Wv.rearrange("(k p) m -> p k m", p=P))
        wo_sb = consts.tile([P, CB, C], MM_DT)
        nc.sync.dma_start(wo_sb, Wo.rearrange("(k p) m -> p k m", p=P))
        bq_sb = consts.tile([P, CB], F32)
        nc.sync.dma_start(bq_sb, bq.rearrange("(m p) -> p m", p=P))
        bk_sb = consts.tile([P, CB], F32)
        nc.sync.dma_start(bk_sb, bk.rearrange("(m p) -> p m", p=P))

        # ------------------------------------------------------------------
        # Software pipeline: each batch-element pair's DENSE work (proj/V,
        # big N=512 matmuls) is emitted interleaved with the PREVIOUS pair's
        # SPARSE work (attention: small matmuls gated on softmax round
        # trips).  This keeps PE activity dense everywhere so the HAM clock
        # gate stays at full rate.
        # ------------------------------------------------------------------

        def make_dense(pi):
            """Thunks for proj+V of pair pi. Returns (thunks, state)."""
            state = {}

            def load_x():
                xT = work.tile([P, CB, 2 * N], MM_DT, tag="xT", name="xT")
                for e in range(2):
                    i = pi * 2 + e
                    nc.sync.dma_start(
                        xT[:, :, e * N : (e + 1) * N],
                        xt_in[i].rearrange("(cb p) n -> p cb n", p=P),
                    )
                state["xT"] = xT
                state["qt"] = work.tile([P, CB, 2 * N], MM_DT, tag="qt", name="qt")
                state["kt"] = work.tile([P, CB, 2 * N], MM_DT, tag="kt", name="kt")
                state["v"] = [
                    work.tile([P, TB, C], PV_DT, tag=f"v{e}", name=f"v{e}")
                    for e in range(2)
                ]

            def proj(mb, which):
                xT = state["xT"]
                w_sb, b_sb, out = (
                    (wq_sb, bq_sb, state["qt"])
                    if which == "q"
                    else (wk_sb, bk_sb, state["kt"])
                )
                pq = ps_mm.tile([P, 2 * N], F32, tag="mm", name="pq")
                for k in range(CB):
                    nc.tensor.matmul(
                        pq,
                        w_sb[:, k, mb * P : (mb + 1) * P],
                        xT[:, k, :],
                        start=(k == 0),
                        stop=(k == CB - 1),
                    )
                nc.vector.tensor_scalar_add(
                    out[:, mb, :], pq, b_sb[:, mb : mb + 1]
                )

            def vproj(e, t):
                xT = state["xT"]
                eo = e * N
                pv = ps_mm.tile([P, C], F32, tag="mm", name="pv")
                for k in range(CB):
                    nc.tensor.matmul(
                        pv,
                        xT[:, k, eo + t * P : eo + (t + 1) * P],
                        wv_sb[:, k, :],
                        start=(k == 0),
                        stop=(k == CB - 1),
                    )
                nc.vector.tensor_copy(state["v"][e][:, t, :], pv)

            thunks = [load_x]
            for mb in range(CB):
                thunks.append(lambda mb=mb: proj(mb, "q"))
                thunks.append(lambda mb=mb: proj(mb, "k"))
            for e in range(2):
                for t in range(TB):
                    thunks.append(lambda e=e, t=t: vproj(e, t))
            return thunks, state

        def make_sparse(pi, state):
            """Thunks for attention + out-proj of pair pi."""
            qt, kt = state["qt"], state["kt"]
            at_sbs = [
                work.tile([P, CB, N], MM_DT, tag=f"at{e}", name=f"at_sb{e}")
                for e in range(2)
            ]

            def attn(hp_i, e):
                eo = e * N
                v_sb = state["v"][e]
                # ST blocks, one 1-bank PSUM tile per head:
                # s[:, 0:2, :] = K_h(k0)^T Q_h  (q0 | q1)
                # s[:, 2, :]   = K_h(k1)^T Q_h  (q1)
                ests = []
                for sub in range(2):
                    hp = D * sub
                    qh = qt[hp : hp + D, hp_i, eo : eo + N]
                    kh = kt[hp : hp + D, hp_i, eo : eo + N]
                    s = ps_sc.tile([P, 4, P], F32, tag="sc", name="s")
                    nc.tensor.matmul(
                        s[:, 0:2, :], kh[:, 0:P], qh,
                        start=True, stop=True, skip_group_check=True,
                    )
                    nc.tensor.matmul(
                        s[:, 2, :], kh[:, P:N], qh[:, P:N],
                        start=True, stop=True, skip_group_check=True,
                    )
                    # exp((S+mask)/sqrt(d)) without mask: masked lanes are
                    # zeroed right after (reference underflows to 0 too).
                    est = heads.tile([P, 3, P], PV_DT, tag="est", name="est")
                    nc.scalar.activation(
                        est, s[:, 0:3, :], AF.Exp, scale=0.125
                    )
                    nc.gpsimd.tensor_mul(
                        est[:, 0:3:2, :], est[:, 0:3:2, :], mk4[:, 0, :, :]
                    )
                    ests.append(est)
                # Z column sums (ones-matmul) + attnT_raw, col-packed,
                # sharing one PSUM bank ([:, 0, :] = attnT, [:, 1, :] = Z).
                zat = ps_zat.tile([P, 2, N], F32, tag="zat", name="zat")
                for sub in range(2):
                    hp = D * sub
                    nc.tensor.matmul(
                        zat[hp : hp + D, 1, :],
                        ones64,
                        ests[sub][:, 0:2, :],
                        start=True, stop=False, skip_group_check=True,
                    )
                    nc.tensor.matmul(
                        zat[hp : hp + D, 1, P:N],
                        ones64,
                        ests[sub][:, 2, :],
                        start=False, stop=True, skip_group_check=True,
                    )
                for sub in range(2):
                    h = hp_i * 2 + sub
                    hp = D * sub
                    nc.tensor.matmul(
                        zat[hp : hp + D, 0, :],
                        v_sb[:, 0, h * D : (h + 1) * D],
                        ests[sub][:, 0:2, :],
                        start=True, stop=False, skip_group_check=True,
                    )
                    nc.tensor.matmul(
                        zat[hp : hp + D, 0, P:N],
                        v_sb[:, 1, h * D : (h + 1) * D],
                        ests[sub][:, 2, :],
                        start=False, stop=True, skip_group_check=True,
                    )
                # 1/Z = exp(-ln Z): both funcs live in one ACT table (the
                # build pins it), so no InstReciprocal and no table thrash.
                lz = heads.tile([P, N], F32, tag="lz", name="lz")
                nc.scalar.activation(lz, zat[:, 1, :], AF.Ln)
                rz = heads.tile([P, N], F32, tag="rz", name="rz")
                nc.scalar.activation(rz, lz, AF.Exp, scale=-1.0)
                nc.vector.tensor_mul(at_sbs[e][:, hp_i, :], zat[:, 0, :], rz)

            def outproj(e, t):
                i = pi * 2 + e
                py = ps_mm.tile([P, C], F32, tag="mm", name="py")
                for k in range(CB):
                    nc.tensor.matmul(
                        py,
                        at_sbs[e][:, k, t * P : (t + 1) * P],
                        wo_sb[:, k, :],
                        start=(k == 0),
                        stop=(k == CB - 1),
                    )
                y_sb = io.tile([P, C], F32, tag="y", name="y_sb")
                nc.vector.tensor_copy(y_sb, py)
                nc.sync.dma_start(
                    y[i].rearrange("(t p) c -> p t c", p=P)[:, t, :], y_sb
                )

            thunks = []
            for hp_i in range(H // 2):
                for e in range(2):
                    thunks.append(lambda hp_i=hp_i, e=e: attn(hp_i, e))
            for e in range(2):
                for t in range(TB):
                    thunks.append(lambda e=e, t=t: outproj(e, t))
            return thunks

        prev_sparse = []
        for pi in range(nb // 2):
            dense, state = make_dense(pi)
            # interleave: dense thunks of pair pi with sparse of pair pi-1
            n = max(len(dense), len(prev_sparse))
            for j in range(n):
                if j < len(dense):
                    dense[j]()
                if j < len(prev_sparse):
                    prev_sparse[j]()
            prev_sparse = make_sparse(pi, state)
        for t in prev_sparse:
            t()

    return nc


_NC_CACHE: dict = {}


class _PinnedActBacc(bacc.Bacc):
    """Bacc that pins every activation to the one act-func table holding
    both exp and ln ("natural_log_exp_and_others"), so the 1/Z = exp(-ln Z)
    path doesn't thrash 1283ns ACT_TABLE_LOADs between exp and ln tables.
    Table ids stay indices into the unmodified act_info.json, so execution
    is unchanged -- this only steers the compile-time table choice."""

    def insert_act_table_loads(self):
        import bass_rust as _bass_rust
        from concourse.hw_specs import get_activation_tables

        has_activation = any(
            isinstance(i, mybir.InstActivation)
            for b in self.main_func.blocks
            for i in b.instructions
        )
        if not has_activation:
            return
        pin = {AF.Exp, AF.Ln, AF.Identity, AF.Copy}
        tables = []
        for name, funcs in get_activation_tables(self.m.arch).items():
            if name != "natural_log_exp_and_others":
                funcs = funcs - pin
            tables.append((name, funcs))
        _bass_rust.insert_act_table_loads(self, tables)


def _build(nb: int = NB) -> bass.Bass:
    key = nb
    if key not in _NC_CACHE:
        nc = _PinnedActBacc()
        _emit(nc, nb)
        nc.finalize()
        _NC_CACHE[key] = nc
    return _NC_CACHE[key]


def _run(inputs: dict, nb: int = NB, trace: bool = False):
    """Returns (y_full [8*nb, N, C], BassKernelResults)."""
    from concourse.bass_utils import run_bass_kernel_spmd

    import ml_dtypes

    bf16 = ml_dtypes.bfloat16
    x = np.asarray(inputs["x"], np.float32)[: NCORES * nb]
    xt = np.ascontiguousarray(x.transpose(0, 2, 1)).astype(bf16)
    Wq = np.ascontiguousarray(np.asarray(inputs["Wq"], np.float32).astype(bf16))
    Wk = np.ascontiguousarray(np.asarray(inputs["Wk"], np.float32).astype(bf16))
    Wv = np.ascontiguousarray(np.asarray(inputs["Wv"], np.float32).astype(bf16))
    Wo = np.ascontiguousarray(np.asarray(inputs["Wo"], np.float32).astype(bf16))
    bq = np.ascontiguousarray(np.asarray(inputs["bq"], np.float32))
    bk = np.ascontiguousarray(np.asarray(inputs["bk"], np.float32))
    bv = np.asarray(inputs["bv"], np.float32)
    bo = np.asarray(inputs["bo"], np.float32)

    nc = _build(nb)
    in_maps = [
        {
            "xt": np.ascontiguousarray(xt[c * nb : (c + 1) * nb]),
            "Wq": Wq,
            "Wk": Wk,
            "Wv": Wv,
            "Wo": Wo,
            "bq": bq,
            "bk": bk,
        }
        for c in range(NCORES)
    ]
    res = run_bass_kernel_spmd(nc, in_maps, list(range(NCORES)), trace=trace)
    y = np.concatenate([r["y"] for r in res.results], axis=0)
    # host-side fold of bv/bo (exact: softmax rows sum to 1)
    y = y + (bv @ np.asarray(inputs["Wo"], np.float32) + bo)
    return y, res


def kernel(**inputs) -> np.ndarray:
    y, _ = _run(inputs, nb=NB, trace=False)
    return y.astype(np.float32)


# revision 22
# speedup vs baseline: 1.9277x; 1.0121x over previous
"""Trainium2 Bass kernel for a causal multi-head attention block.

Problem: y = MHA(x), B=256, N=256 (seq), C=512, H=8 heads, d=64
  Q = x@Wq + bq ; K = x@Wk + bk ; V = x@Wv + bv   (per-head split)
  S = Q K^T ; scaled = (S + causal_mask*-1e5)/sqrt(d) ; P = softmax(scaled)
  y = (P V merged) @ Wo + bo

Sharding: pure data-parallel over batch B across 8 NeuronCores (32 batch
elements per core); weights replicated; no collectives.

Host-side marshalling inside kernel():
 - x is pre-transposed to xT [b, C, N] and cast to bf16.
 - weights pre-cast to bf16.
 - bv/bo folded on host: softmax rows sum to 1, so V's bias contributes
   attn@(1 bv^T) = bv exactly, hence y += bv@Wo + bo after gathering.

Device pipeline (transpose-free attention):
 - QT/KT [C, 2N] = W^T @ xT computed per batch-element pair; bq/bk fused
   into the PSUM->SBUF evacuation (DVE tensor_scalar add, per-partition).
 - V [N, C] = x @ Wv per element (DVE evac).
 - Scores are computed TRANSPOSED: ST[k, q], keys on partitions, by
   swapping the matmul operands (lhsT=K_h, rhs=Q_h). Only the 3
   causally-live 128x128 blocks are computed:
     blk0 = (k0, q0), blk1 = (k0, q1), blk2 = (k1, q1).
   Two heads run concurrently in disjoint 64-row PE groups.
 - exp via one ACT op per head (scale=1/8, PSUM->SBUF bf16, no max-sub:
   scores are O(1)). Causal masking = multiply the two diagonal blocks by
   a 0/1 lower-triangular bf16 constant on GPSIMD (masked reference lanes
   underflow exp->0 exactly, so 0 matches bit-for-bit).
 - Z[q] (softmax denominators) via ones-matmul column sums on the PE into
   the same PSUM bank as attnT_raw; head pairs col-packed.
 - attnT_raw [d, q] = V_h^T @ expST_h straight into the [C, N] layout
   (col-packed pairs). Normalization happens on the small attnT tile:
   at_sb = at_raw * (1/Z), with 1/Z = exp(-ln Z) on the ACT engine -- the
   build pins all activations to the one act-func table containing both
   exp and ln, so there is no 1283ns table swap and no slow DVE
   InstReciprocal. The 1/Z broadcast over d rows comes for free since the
   ones-matmul replicates Z across all 64 output rows per head.
 - y [N, C] = attnT^T @ Wo, DVE evac, DMA out.

The whole schedule is software-pipelined: each pair's dense projection
matmuls (N=512, streaming-roofline) are emitted interleaved with the
previous pair's sparse attention work, so the tensor engine never idles
long enough for the HAM clock gate to re-throttle (throttle_active
dropped from 250us to <10us) and softmax round trips hide behind
projection streams.

No PE transposes anywhere -> tensor engine stays HAM-warm and does only
real matmul work.
"""

import os
import sys

sys.path.insert(0, "/opt/trn_rl_repo")

import numpy as np

import concourse.bass as bass
import concourse.mybir as mybir
import concourse.tile as tile
from concourse import bacc

B, N, C, H, D = 256, 256, 512, 8, 64
NCORES = 8
NB = B // NCORES  # batch elements per core
P = 128
F32 = mybir.dt.float32

AF = mybir.ActivationFunctionType

MM_DT = mybir.dt.bfloat16
PV_DT = mybir.dt.bfloat16


def _emit(nc: bass.Bass, nb: int):
    xt_in = nc.dram_tensor("xt", [nb, C, N], MM_DT, kind="ExternalInput")
    Wq = nc.dram_tensor("Wq", [C, C], MM_DT, kind="ExternalInput")
    Wk = nc.dram_tensor("Wk", [C, C], MM_DT, kind="ExternalInput")
    Wv = nc.dram_tensor("Wv", [C, C], MM_DT, kind="ExternalInput")
    Wo = nc.dram_tensor("Wo", [C, C], MM_DT, kind="ExternalInput")
    bq = nc.dram_tensor("bq", [C], F32, kind="ExternalInput")
    bk = nc.dram_tensor("bk", [C], F32, kind="ExternalInput")
    y = nc.dram_tensor("y", [nb, N, C], F32, kind="ExternalOutput")

    CB = C // P  # 4 column blocks of 128
    TB = N // P  # 2 row blocks of 128

    with (
        tile.TileContext(nc) as tc,
        tc.tile_pool(name="consts", bufs=1) as consts,
        tc.tile_pool(name="io", bufs=3) as io,
        tc.tile_pool(name="work", bufs=3) as work,
        tc.tile_pool(name="heads", bufs=int(os.environ.get("HB", "8"))) as heads,
        tc.tile_pool(name="ps_mm", bufs=int(os.environ.get("PS_MM", "3")), space="PSUM") as ps_mm,
        tc.tile_pool(name="ps_sc", bufs=int(os.environ.get("PS_SC", "3")), space="PSUM") as ps_sc,
        tc.tile_pool(name="ps_zat", bufs=int(os.environ.get("PS_ZAT", "2")), space="PSUM") as ps_zat,
    ):
        # ---- constants ----
        # keep-mask for the two causal diagonal blocks of both heads of a
        # pair: 1 where k <= q (p >= f), else 0.  Layout [P, sub, blk, P].
        mk4 = consts.tile([P, 2, 2, P], PV_DT)
        nc.gpsimd.memset(mk4, 1.0)
        for i in range(2):
            for j in range(2):
                nc.gpsimd.affine_select(
                    out=mk4[:, i, j, :],
                    in_=mk4[:, i, j, :],
                    compare_op=mybir.AluOpType.is_ge,
                    fill=0.0,
                    base=0,
                    pattern=[[1, P]],
                    channel_multiplier=-1,
                )
        ones64 = consts.tile([P, D], PV_DT)
        nc.gpsimd.memset(ones64, 1.0)

        wq_sb = consts.tile([P, CB, C], MM_DT)
        wk_sb = consts.tile([P, CB, C], MM_DT)
        wv_sb = consts.tile([P, CB, C], MM_DT)
        wo_sb = consts.tile([P, CB, C], MM_DT)
        bq_sb = consts.tile([P, CB], F32)
        bk_sb = consts.tile([P, CB], F32)

        # ------------------------------------------------------------------
        # Software pipeline: each batch-element pair's DENSE work (proj/V,
        # big N=512 matmuls) is emitted interleaved with the PREVIOUS pair's
        # SPARSE work (attention: small matmuls gated on softmax round
        # trips).  This keeps PE activity dense everywhere so the HAM clock
        # gate stays at full rate.
        # ------------------------------------------------------------------

        def make_dense(pi):
            """Thunks for proj+V of pair pi. Returns (thunks, state)."""
            state = {}

            def load_x():
                xT = work.tile([P, CB, 2 * N], MM_DT, tag="xT", name="xT")
                for cb in range(CB):
                    for e in range(2):
                        i = pi * 2 + e
                        nc.sync.dma_start(
                            xT[:, cb, e * N : (e + 1) * N],
                            xt_in[i].rearrange("(cb p) n -> p cb n", p=P)[:, cb, :],
                        )
                state["xT"] = xT
                if pi == 0:
                    # weight loads deferred behind the first xT, chunked per
                    # k-block (subtile deps let matmuls start on partial
                    # data), and spread across idle engine queues so DMA
                    # descriptor generation doesn't serialize on one queue.
                    for k in range(CB):
                        nc.scalar.dma_start(
                            wq_sb[:, k, :],
                            Wq.rearrange("(k p) m -> p k m", p=P)[:, k, :],
                        )
                    nc.scalar.dma_start(bq_sb, bq.rearrange("(m p) -> p m", p=P))
                    for k in range(CB):
                        nc.scalar.dma_start(
                            wk_sb[:, k, :],
                            Wk.rearrange("(k p) m -> p k m", p=P)[:, k, :],
                        )
                    nc.scalar.dma_start(bk_sb, bk.rearrange("(m p) -> p m", p=P))
                    nc.gpsimd.dma_start(wv_sb, Wv.rearrange("(k p) m -> p k m", p=P))
                    nc.gpsimd.dma_start(wo_sb, Wo.rearrange("(k p) m -> p k m", p=P))
                state["qt"] = work.tile([P, CB, 2 * N], MM_DT, tag="qt", name="qt")
                state["kt"] = work.tile([P, CB, 2 * N], MM_DT, tag="kt", name="kt")
                state["v"] = [
                    work.tile([P, TB, C], PV_DT, tag=f"v{e}", name=f"v{e}")
                    for e in range(2)
                ]

            def proj(mb, which):
                xT = state["xT"]
                w_sb, b_sb, out = (
                    (wq_sb, bq_sb, state["qt"])
                    if which == "q"
                    else (wk_sb, bk_sb, state["kt"])
                )
                pq = ps_mm.tile([P, 2 * N], F32, tag="mm", name="pq")
                for k in range(CB):
                    nc.tensor.matmul(
                        pq,
                        w_sb[:, k, mb * P : (mb + 1) * P],
                        xT[:, k, :],
                        start=(k == 0),
                        stop=(k == CB - 1),
                    )
                nc.vector.tensor_scalar_add(
                    out[:, mb, :], pq, b_sb[:, mb : mb + 1]
                )

            def vproj(e, t):
                xT = state["xT"]
                eo = e * N
                pv = ps_mm.tile([P, C], F32, tag="mm", name="pv")
                for k in range(CB):
                    nc.tensor.matmul(
                        pv,
                        xT[:, k, eo + t * P : eo + (t + 1) * P],
                        wv_sb[:, k, :],
                        start=(k == 0),
                        stop=(k == CB - 1),
                    )
                nc.vector.tensor_copy(state["v"][e][:, t, :], pv)

            thunks = [load_x]
            for mb in range(CB):
                thunks.append(lambda mb=mb: proj(mb, "q"))
                thunks.append(lambda mb=mb: proj(mb, "k"))
            for e in range(2):
                for t in range(TB):
                    thunks.append(lambda e=e, t=t: vproj(e, t))
            return thunks, state

        def make_sparse(pi, state):
            """Thunks for attention + out-proj of pair pi."""
            qt, kt = state["qt"], state["kt"]
            at_sbs = [
                work.tile([P, CB, N], MM_DT, tag=f"at{e}", name=f"at_sb{e}")
                for e in range(2)
            ]

            def attn(hp_i, e):
                eo = e * N
                v_sb = state["v"][e]
                # ST blocks, one 1-bank PSUM tile per head:
                # s[:, 0:2, :] = K_h(k0)^T Q_h  (q0 | q1)
                # s[:, 2, :]   = K_h(k1)^T Q_h  (q1)
                ests = []
                for sub in range(2):
                    hp = D * sub
                    qh = qt[hp : hp + D, hp_i, eo : eo + N]
                    kh = kt[hp : hp + D, hp_i, eo : eo + N]
                    s = ps_sc.tile([P, 4, P], F32, tag="sc", name="s")
                    nc.tensor.matmul(
                        s[:, 0:2, :], kh[:, 0:P], qh,
                        start=True, stop=True, skip_group_check=True,
                    )
                    nc.tensor.matmul(
                        s[:, 2, :], kh[:, P:N], qh[:, P:N],
                        start=True, stop=True, skip_group_check=True,
                    )
                    # exp((S+mask)/sqrt(d)) without mask: masked lanes are
                    # zeroed right after (reference underflows to 0 too).
                    est = heads.tile([P, 3, P], PV_DT, tag="est", name="est")
                    nc.scalar.activation(
                        est, s[:, 0:3, :], AF.Exp, scale=0.125
                    )
                    nc.gpsimd.tensor_mul(
                        est[:, 0:3:2, :], est[:, 0:3:2, :], mk4[:, 0, :, :]
                    )
                    ests.append(est)
                # Z column sums (ones-matmul) + attnT_raw, col-packed,
                # sharing one PSUM bank ([:, 0, :] = attnT, [:, 1, :] = Z).
                zat = ps_zat.tile([P, 2, N], F32, tag="zat", name="zat")
                for sub in range(2):
                    hp = D * sub
                    nc.tensor.matmul(
                        zat[hp : hp + D, 1, :],
                        ones64,
                        ests[sub][:, 0:2, :],
                        start=True, stop=False, skip_group_check=True,
                    )
                    nc.tensor.matmul(
                        zat[hp : hp + D, 1, P:N],
                        ones64,
                        ests[sub][:, 2, :],
                        start=False, stop=True, skip_group_check=True,
                    )
                for sub in range(2):
                    h = hp_i * 2 + sub
                    hp = D * sub
                    nc.tensor.matmul(
                        zat[hp : hp + D, 0, :],
                        v_sb[:, 0, h * D : (h + 1) * D],
                        ests[sub][:, 0:2, :],
                        start=True, stop=False, skip_group_check=True,
                    )
                    nc.tensor.matmul(
                        zat[hp : hp + D, 0, P:N],
                        v_sb[:, 1, h * D : (h + 1) * D],
                        ests[sub][:, 2, :],
                        start=False, stop=True, skip_group_check=True,
                    )
                # 1/Z = exp(-ln Z): both funcs live in one ACT table (the
                # build pins it), so no InstReciprocal and no table thrash.
                lz = heads.tile([P, N], F32, tag="lz", name="lz")
                nc.scalar.activation(lz, zat[:, 1, :], AF.Ln)
                rz = heads.tile([P, N], F32, tag="rz", name="rz")
                nc.scalar.activation(rz, lz, AF.Exp, scale=-1.0)
                nc.vector.tensor_mul(at_sbs[e][:, hp_i, :], zat[:, 0, :], rz)

            def outproj(e, t):
                i = pi * 2 + e
                py = ps_mm.tile([P, C], F32, tag="mm", name="py")
                for k in range(CB):
                    nc.tensor.matmul(
                        py,
                        at_sbs[e][:, k, t * P : (t + 1) * P],
                        wo_sb[:, k, :],
                        start=(k == 0),
                        stop=(k == CB - 1),
                    )
                y_sb = io.tile([P, C], F32, tag="y", name="y_sb")
                nc.vector.tensor_copy(y_sb, py)
                nc.sync.dma_start(
                    y[i].rearrange("(t p) c -> p t c", p=P)[:, t, :], y_sb
                )

            thunks = []
            for e in range(2):
                for hp_i in range(H // 2):
                    thunks.append(lambda hp_i=hp_i, e=e: attn(hp_i, e))
                for t in range(TB):
                    thunks.append(lambda e=e, t=t: outproj(e, t))
            return thunks

        prev_sparse = []
        for pi in range(nb // 2):
            dense, state = make_dense(pi)
            # interleave: dense thunks of pair pi with sparse of pair pi-1
            n = max(len(dense), len(prev_sparse))
            for j in range(n):
                if j < len(dense):
                    dense[j]()
                if j < len(prev_sparse):
                    prev_sparse[j]()
            prev_sparse = make_sparse(pi, state)
        for t in prev_sparse:
            t()

    return nc


_NC_CACHE: dict = {}


class _PinnedActBacc(bacc.Bacc):
    """Bacc that pins every activation to the one act-func table holding
    both exp and ln ("natural_log_exp_and_others"), so the 1/Z = exp(-ln Z)
    path doesn't thrash 1283ns ACT_TABLE_LOADs between exp and ln tables.
    Table ids stay indices into the unmodified act_info.json, so execution
    is unchanged -- this only steers the compile-time table choice."""

    def insert_act_table_loads(self):
        import bass_rust as _bass_rust
        from concourse.hw_specs import get_activation_tables

        has_activation = any(
            isinstance(i, mybir.InstActivation)
            for b in self.main_func.blocks
            for i in b.instructions
        )
        if not has_activation:
            return
        pin = {AF.Exp, AF.Ln, AF.Identity, AF.Copy}
        tables = []
        for name, funcs in get_activation_tables(self.m.arch).items():
            if name != "natural_log_exp_and_others":
                funcs = funcs - pin
            tables.append((name, funcs))
        _bass_rust.insert_act_table_loads(self, tables)


def _build(nb: int = NB) -> bass.Bass:
    key = nb
    if key not in _NC_CACHE:
        nc = _PinnedActBacc()
        _emit(nc, nb)
        nc.finalize()
        _NC_CACHE[key] = nc
    return _NC_CACHE[key]


def _run(inputs: dict, nb: int = NB, trace: bool = False):
    """Returns (y_full [8*nb, N, C], BassKernelResults)."""
    from concourse.bass_utils import run_bass_kernel_spmd

    import ml_dtypes

    bf16 = ml_dtypes.bfloat16
    x = np.asarray(inputs["x"], np.float32)[: NCORES * nb]
    xt = np.ascontiguousarray(x.transpose(0, 2, 1)).astype(bf16)
    Wq = np.ascontiguousarray(np.asarray(inputs["Wq"], np.float32).astype(bf16))
    Wk = np.ascontiguousarray(np.asarray(inputs["Wk"], np.float32).astype(bf16))
    Wv = np.ascontiguousarray(np.asarray(inputs["Wv"], np.float32).astype(bf16))
    Wo = np.ascontiguousarray(np.asarray(inputs["Wo"], np.float32).astype(bf16))
    bq = np.ascontiguousarray(np.asarray(inputs["bq"], np.float32))
    bk = np.ascontiguousarray(np.asarray(inputs["bk"], np.float32))
    bv = np.asarray(inputs["bv"], np.float32)
    bo = np.asarray(inputs["bo"], np.float32)

    nc = _build(nb)
    in_maps = [
        {
            "xt": np.ascontiguousarray(xt[c * nb : (c + 1) * nb]),
            "Wq": Wq,
            "Wk": Wk,
            "Wv": Wv,
            "Wo": Wo,
            "bq": bq,
            "bk": bk,
        }
        for c in range(NCORES)
    ]
    res = run_bass_kernel_spmd(nc, in_maps, list(range(NCORES)), trace=trace)
    y = np.concatenate(
        [np.asarray(r["y"], np.float32) for r in res.results], axis=0
    )
    # host-side fold of bv/bo (exact: softmax rows sum to 1)
    y = y + (bv @ np.asarray(inputs["Wo"], np.float32) + bo)
    return y, res


def kernel(**inputs) -> np.ndarray:
    y, _ = _run(inputs, nb=NB, trace=False)
    return y.astype(np.float32)
